# revision 28
# baseline (speedup 1.0000x reference)
"""Distributed causal-attention kernel for TRN2 (8 NeuronCores).

Module: qkv = x@w_attn+b; q,k l2-normalized per head; scaled (8.0) causal
softmax attention; out = (attn@v reassembled)@w_proj + b_proj.
Shapes: x [2,2048,1024], 16 heads x 64 dim.

Sharding: pure tensor-parallel over heads (2 heads/core).  Each core
computes qkv for its heads over the full batch*seq, runs attention, then two
8-core AllToAlls (one per head, pipelined against compute) redistribute the
per-head outputs to row-shards so each core applies the full output
projection to its 512 rows.

Key device-side choices:
 - host passes x transposed; qkv lands in [seq, cols] layout where q,k are
   normalized with free-axis norms (all per-tile norm stats batched so ACT
   loads the Ln/Exp table sets once instead of thrashing per tile)
 - q,k are PE-transposed into per-HEAD tensors with BATCH on the partition
   axis (b0 rows 0:64, b1 rows 64:128) so the two batches' K=64 score
   matmuls row-tile into disjoint PE row-groups and run concurrently
 - scores are computed transposed [k, q]; the exp'd tile is directly the
   AV matmul's stationary operand; the softmax denominator comes from a
   ones column appended to v
 - each q-subtile accumulates in its own PSUM bank (matmul start=True
   clears has_written for its whole 2KB zero region, so concurrent
   accumulation groups must never share a bank); subtiles are processed in
   two passes of 4 accumulators to stay within 8 banks
 - o is transposed on-device pre-A2A so the collective payload is oT and
   the receive side is a single plain DMA into the projection layout
"""
import sys

if '/opt/trn_rl_repo' not in sys.path:
    sys.path.insert(0, '/opt/trn_rl_repo')

import numpy as np
import ml_dtypes

import concourse.bass as bass
import concourse.mybir as mybir
from concourse import bacc, tile
from concourse.bass import ts, ds
from concourse.bass_utils import run_bass_kernel_spmd
from concourse.masks import make_identity

B, S, D, H = 2, 2048, 1024, 16
HD = D // H                 # 64
NCORES = 8
HPC = H // NCORES           # 2 heads per core
SEQT = 128
NT = (B * S) // SEQT        # 32 seq tiles (batch-major)
TPB = S // SEQT             # 16 tiles per batch
QSPAN = 512
NSPAN = S // QSPAN          # 4 q-spans per batch
ROWS = (B * S) // NCORES    # 512 output rows per core
KC = D // 128               # 8 contraction chunks
W3 = 3 * HPC * HD           # 384 qkv columns per core
BF = mybir.dt.bfloat16
F32 = mybir.dt.float32
HALF_LN8 = 1.0397207708399179  # 0.5*ln(8): folds the 8.0 score scale
AF = mybir.ActivationFunctionType
MUL = mybir.AluOpType.mult


def build(dbg=False):
    nc = bacc.Bacc("TRN2", target_bir_lowering=False, debug=False,
                   num_devices=NCORES)
    xt = nc.dram_tensor("xt", [D, B * S], BF, kind="ExternalInput")
    wq = nc.dram_tensor("wq", [D, W3], BF, kind="ExternalInput")
    ba = nc.dram_tensor("ba", [1, W3], BF, kind="ExternalInput")
    wp = nc.dram_tensor("wp", [D, D], BF, kind="ExternalInput")
    bp = nc.dram_tensor("bp", [1, D], BF, kind="ExternalInput")
    out = nc.dram_tensor("out", [ROWS, D], F32, kind="ExternalOutput")
    if dbg:
        d_qt = nc.dram_tensor("d_qt", [128, HPC * S], BF, kind="ExternalOutput")
        d_kt = nc.dram_tensor("d_kt", [128, HPC * S], BF, kind="ExternalOutput")
        d_v = nc.dram_tensor("d_v", [128, NT * 2 * (HD + 1)], BF,
                             kind="ExternalOutput")
        d_oc = nc.dram_tensor("d_oc", [128, NCORES * ROWS], BF,
                              kind="ExternalOutput")

    with tile.TileContext(nc) as tc:
        with tc.tile_pool(name="persist", bufs=1) as pp, \
             tc.tile_pool(name="dram", bufs=1, space="DRAM") as dram, \
             tc.tile_pool(name="work", bufs=4) as work:

            # ---- persistent SBUF ----
            wq_sb = pp.tile([128, KC, W3], BF, name="wq_sb")
            wp_sb = pp.tile([128, KC, D], BF, name="wp_sb")
            ba_sb = pp.tile([1, W3], BF, name="ba_sb")
            bp_sb = pp.tile([1, D], BF, name="bp_sb")
            ones_sb = pp.tile([1, 128], BF, name="ones_sb")
            ones_f32 = pp.tile([1, HD], F32, name="ones_f32")
            c_bias = pp.tile([128, 1], F32, name="c_bias")
            c_scale = pp.tile([128, 1], F32, name="c_scale")
            ident = pp.tile([128, 128], BF, name="ident")
            tri = pp.tile([128, 128], BF, name="tri")
            # q,k working copies (normalized in place) + batched norm stats
            qk_all = pp.tile([128, NT, 2 * HPC * HD], BF, name="qk_all")
            n2_all = pp.tile([128, NT, 2 * HPC], F32, name="n2_all")
            rn_all = pp.tile([128, NT, 2 * HPC], F32, name="rn_all")
            # qT/kT per HEAD: batch0 rows 0:64, batch1 rows 64:128
            QT = [pp.tile([128, S], BF, name=f"qt{h}") for h in range(HPC)]
            KT = [pp.tile([128, S], BF, name=f"kt{h}") for h in range(HPC)]
            # v in [seq, hd] layout, per head augmented with a ones column
            v_sb = pp.tile([128, NT, 2 * (HD + 1)], BF, name="v_sb")
            ocT = pp.tile([128, NCORES, ROWS], BF, name="ocT")

            # per-head A2A payload is oT: shard p = rows [64p:64p+64] = this
            # core's head-h oT columns destined for core p
            a2a_in = [dram.tile([64 * NCORES, ROWS], BF, name=f"a2a_in{h}")
                      for h in range(HPC)]
            a2a_out = [dram.tile([64 * NCORES, ROWS], BF, name=f"a2a_out{h}")
                       for h in range(HPC)]

            # ---- constants (ordered so phase B can start early) ----
            for kc in range(KC):
                nc.sync.dma_start(wq_sb[:, kc, :], wq[ts(kc, 128), :])
            nc.sync.dma_start(ba_sb[:], ba[:])
            nc.sync.dma_start(bp_sb[:], bp[:])
            nc.gpsimd.memset(ones_sb[:], 1.0)
            nc.gpsimd.memset(ones_f32[:], 1.0)
            nc.gpsimd.memset(c_bias[:], HALF_LN8)
            nc.gpsimd.memset(c_scale[:], -0.5)
            nc.gpsimd.memset(v_sb[:], 1.0)
            make_identity(nc, ident[:])
            # tri[k, q] = 1 where q >= k (valid causal), else 0
            nc.gpsimd.memset(tri[:], 1.0)
            nc.gpsimd.affine_select(
                out=tri[:], in_=tri[:], compare_op=mybir.AluOpType.is_ge,
                fill=0.0, base=0, pattern=[[1, 128]], channel_multiplier=-1)

            # ---- phase B1: qkv matmuls + norm stats ----
            # xt lives only here; scoping frees its 64KB/partition for the
            # attention-phase e-tile pool
            with tc.tile_pool(name="xtp", bufs=1) as xtp, \
                 tc.tile_pool(name="ps_qkv", bufs=2, space="PSUM") as ps_qkv:
                xt_sb = xtp.tile([128, KC, B * S], BF, name="xt_sb")
                for tq in range(4):
                    for kc in range(KC):
                        nc.sync.dma_start(xt_sb[:, kc, ds(1024 * tq, 1024)],
                                          xt[ts(kc, 128), ds(1024 * tq, 1024)])
                for kc in range(KC):
                    nc.sync.dma_start(wp_sb[:, kc, :], wp[ts(kc, 128), :])
                for t in range(NT):
                    ps = ps_qkv.tile([128, W3], F32, tag="ps", name=f"ps{t}")
                    for kc in range(KC):
                        nc.tensor.matmul(ps[:], lhsT=xt_sb[:, kc, ts(t, 128)],
                                         rhs=wq_sb[:, kc, :],
                                         start=(kc == 0), stop=False)
                    nc.tensor.matmul(ps[:], lhsT=ones_sb[:, 0:128], rhs=ba_sb[:],
                                     start=False, stop=True)
                    nc.scalar.copy(qk_all[:, t, :], ps[:, 0:256])
                    nc.scalar.copy(
                        v_sb[:, t, :].rearrange(
                            "p (h e) -> p h e", e=HD + 1)[:, :, 0:HD],
                        ps[:, 256:384].rearrange("p (h e) -> p h e", e=HD))
                    sq = work.tile([128, 2 * HPC * HD], BF, tag="sq",
                                   name=f"sq{t}")
                    nc.vector.tensor_mul(sq[:], qk_all[:, t, :], qk_all[:, t, :])
                    nc.vector.reduce_sum(
                        n2_all[:, t, :], sq[:].rearrange("p (g e) -> p g e", e=HD),
                        axis=mybir.AxisListType.X)

            # batched rnorm = sqrt(8)/sqrt(n2): one Ln + one Exp (one table
            # set load each instead of per-tile thrash)
            nc.scalar.activation(rn_all[:].rearrange("p a b -> p (a b)"),
                                 n2_all[:].rearrange("p a b -> p (a b)"), AF.Ln)
            nc.scalar.activation(rn_all[:].rearrange("p a b -> p (a b)"),
                                 rn_all[:].rearrange("p a b -> p (a b)"), AF.Exp,
                                 scale=c_scale[:], bias=c_bias[:])

            with tc.tile_pool(name="ps_tr", bufs=2, space="PSUM") as ps_tr, \
                 tc.tile_pool(name="psB", bufs=2, space="PSUM") as psB, \
                 tc.tile_pool(name="psC", bufs=4, space="PSUM") as psC, \
                 tc.tile_pool(name="epool", bufs=6) as epool:

                # ---- phase B2: normalize + transpose into per-head
                # batch-packed QT/KT ----
                for t in range(NT):
                    b_, tt = divmod(t, TPB)
                    nc.vector.tensor_tensor(
                        qk_all[:, t, :].rearrange("p (g e) -> p g e", e=HD),
                        qk_all[:, t, :].rearrange("p (g e) -> p g e", e=HD),
                        rn_all[:, t, :, None].broadcast_to([128, 4, HD]), op=MUL)
                    for src0, dst in ((0, QT), (128, KT)):
                        trp = ps_tr.tile([128, 128], BF, tag="tr",
                                         name=f"tr{t}_{src0}")
                        nc.tensor.transpose(
                            trp[:], qk_all[:, t, src0:src0 + 128], ident[:])
                        # rows 0:64 = head0, 64:128 = head1 -> batch-packed
                        # per-head tensors (h0 copy on DVE, h1 on ACT)
                        nc.vector.tensor_copy(
                            dst[0][64 * b_:64 * b_ + 64, ts(tt, 128)],
                            trp[0:64, :])
                        nc.scalar.copy(
                            dst[1][64 * b_:64 * b_ + 64, ts(tt, 128)],
                            trp[64:128, :])

                # ---- phase C: attention (head-major for A2A pipelining;
                # batches row-tiled into concurrent PE row-groups; AV in
                # transposed form: one matmul per k-tile accumulating
                # oT+denom rows, so the result is directly the A2A payload)
                for h in range(HPC):
                    for j in range(NSPAN):
                        nk = 4 * j + 4
                        avT = [psC.tile([HD + 1, QSPAN], F32, tag="av",
                                        name=f"avT{h}_{j}_{b}")
                               for b in range(B)]
                        for i in range(nk):
                            d = i - 4 * j
                            c0 = max(d, 0)
                            for b_ in range(B):
                                sps = psB.tile([128, QSPAN], F32, tag="s",
                                               name=f"s{h}_{j}_{b_}_{i}")
                                nc.tensor.matmul(
                                    sps[:],
                                    lhsT=KT[h][64 * b_:64 * b_ + 64, ts(i, 128)],
                                    rhs=QT[h][64 * b_:64 * b_ + 64,
                                              ds(j * QSPAN, QSPAN)],
                                    start=True, stop=True)
                                e = epool.tile([128, QSPAN], BF, tag="e",
                                               name=f"e{h}_{j}_{b_}_{i}")
                                nc.scalar.activation(e[:, 128 * c0:],
                                                     sps[:, 128 * c0:], AF.Exp)
                                if d >= 0:
                                    nc.vector.tensor_tensor(
                                        e[:, 128 * d:128 * (d + 1)],
                                        e[:, 128 * d:128 * (d + 1)], tri[:],
                                        op=MUL)
                                nc.tensor.matmul(
                                    avT[b_][:, 128 * c0:],
                                    lhsT=v_sb[:, b_ * TPB + i,
                                              65 * h:65 * h + 65],
                                    rhs=e[:, 128 * c0:],
                                    start=(i == 0), stop=(i == nk - 1))
                        for b_ in range(B):
                            rdr = work.tile([1, QSPAN], F32, tag="rdr",
                                            name=f"rdr{h}_{j}_{b_}")
                            nc.vector.reciprocal(rdr[:], avT[b_][HD:HD + 1, :])
                            # broadcast the reciprocal row across 64
                            # partitions via a rank-1 matmul
                            rdb = psC.tile([HD, QSPAN], F32, tag="av",
                                           name=f"rdb{h}_{j}_{b_}")
                            nc.tensor.matmul(rdb[:], lhsT=ones_f32[:],
                                             rhs=rdr[:], start=True, stop=True)
                            osb = work.tile([HD, QSPAN], BF, tag="osb",
                                            name=f"osb{h}_{j}_{b_}")
                            nc.vector.tensor_copy(osb[:], avT[b_][0:HD, :])
                            ot = work.tile([HD, QSPAN], BF, tag="ot",
                                           name=f"ot{h}_{j}_{b_}")
                            nc.vector.tensor_tensor(ot[:], osb[:], rdb[:],
                                                    op=MUL)
                            nc.sync.dma_start(
                                a2a_in[h][ts(4 * b_ + j, 64), :], ot[:])
                    # per-head A2A fires as soon as this head's o is out;
                    # head 0's collective overlaps head 1's attention
                    nc.gpsimd.collective_compute(
                        "AllToAll", mybir.AluOpType.bypass,
                        replica_groups=[list(range(NCORES))],
                        ins=[a2a_in[h][:].opt()], outs=[a2a_out[h][:].opt()])
                    nc.sync.dma_start(
                        ocT[64 * h:64 * h + 64, :, :],
                        a2a_out[h][:].rearrange("(p c) w -> c p w", c=64))

                if dbg:
                    for h in range(HPC):
                        nc.sync.dma_start(d_qt[:, ts(h, S)], QT[h][:])
                        nc.sync.dma_start(d_kt[:, ts(h, S)], KT[h][:])
                    nc.sync.dma_start(d_v[:],
                                      v_sb[:].rearrange("p a b -> p (a b)"))
                    nc.sync.dma_start(
                        d_oc[:], ocT[:].rearrange("p a b -> p (a b)"))

                # ---- phase D: projection ----
                for rt in range(ROWS // 128):
                    for half in range(2):
                        yps = psB.tile([128, 512], F32, tag="s",
                                       name=f"y{rt}_{half}")
                        for p in range(NCORES):
                            nc.tensor.matmul(
                                yps[:], lhsT=ocT[:, p, ts(rt, 128)],
                                rhs=wp_sb[:, p, ds(half * 512, 512)],
                                start=(p == 0), stop=False)
                        nc.tensor.matmul(yps[:], lhsT=ones_sb[:, 0:128],
                                         rhs=bp_sb[:, ds(half * 512, 512)],
                                         start=False, stop=True)
                        ysb = work.tile([128, 512], F32, tag="y", bufs=4,
                                        name=f"ysb{rt}_{half}")
                        nc.vector.tensor_copy(ysb[:], yps[:])
                        nc.sync.dma_start(
                            out[ts(rt, 128), ds(half * 512, 512)], ysb[:])

    nc.compile()
    return nc


_NC = None


def _get_nc():
    global _NC
    if _NC is None:
        _NC = build()
    return _NC


def make_in_maps(x, w_attn, b_attn, w_proj, b_proj):
    bf = ml_dtypes.bfloat16
    xt = np.ascontiguousarray(x.reshape(B * S, D).T).astype(bf)
    wp_ = np.ascontiguousarray(w_proj).astype(bf)
    bp_ = b_proj.reshape(1, D).astype(bf)
    in_maps = []
    for c in range(NCORES):
        sl = slice(128 * c, 128 * c + 128)
        wq_ = np.ascontiguousarray(np.concatenate(
            [w_attn[:, sl], w_attn[:, 1024:2048][:, sl],
             w_attn[:, 2048:3072][:, sl]], axis=1)).astype(bf)
        ba_ = np.concatenate(
            [b_attn[sl], b_attn[1024:2048][sl],
             b_attn[2048:3072][sl]]).reshape(1, W3).astype(bf)
        in_maps.append({"xt": xt, "wq": wq_, "ba": ba_, "wp": wp_, "bp": bp_})
    return in_maps


def gather_out(results):
    out = np.empty((B, S, D), np.float32)
    for c in range(NCORES):
        out[c // 4, ROWS * (c % 4):ROWS * (c % 4 + 1), :] = results[c]["out"]
    return out


def kernel(x, w_attn, b_attn, w_proj, b_proj):
    nc = _get_nc()
    in_maps = make_in_maps(np.asarray(x, np.float32), np.asarray(w_attn, np.float32),
                           np.asarray(b_attn, np.float32),
                           np.asarray(w_proj, np.float32),
                           np.asarray(b_proj, np.float32))
    res = run_bass_kernel_spmd(nc, in_maps, core_ids=list(range(NCORES)))
    return gather_out(res.results)


# revision 29
# speedup vs baseline: 1.0593x; 1.0593x over previous
"""Distributed causal-attention kernel for TRN2 (8 NeuronCores).

Module: qkv = x@w_attn+b; q,k l2-normalized per head; scaled (8.0) causal
softmax attention; out = (attn@v reassembled)@w_proj + b_proj.
Shapes: x [2,2048,1024], 16 heads x 64 dim.

Sharding: pure tensor-parallel over heads (2 heads/core).  Each core
computes qkv for its heads over the full batch*seq, runs attention, then two
8-core AllToAlls (one per head, pipelined against compute) redistribute the
per-head outputs to row-shards so each core applies the full output
projection to its 512 rows.

Key device-side choices:
 - host passes x transposed; qkv lands in [seq, cols] layout where q,k are
   normalized with free-axis norms (all per-tile norm stats batched so ACT
   loads the Ln/Exp table sets once instead of thrashing per tile)
 - q,k are PE-transposed into per-HEAD tensors with BATCH on the partition
   axis (b0 rows 0:64, b1 rows 64:128) so the two batches' K=64 score
   matmuls row-tile into disjoint PE row-groups and run concurrently
 - scores are computed transposed [k, q]; the exp'd tile is directly the
   AV matmul's stationary operand; the softmax denominator comes from a
   ones column appended to v
 - each q-subtile accumulates in its own PSUM bank (matmul start=True
   clears has_written for its whole 2KB zero region, so concurrent
   accumulation groups must never share a bank); subtiles are processed in
   two passes of 4 accumulators to stay within 8 banks
 - o is transposed on-device pre-A2A so the collective payload is oT and
   the receive side is a single plain DMA into the projection layout
"""
import sys

if '/opt/trn_rl_repo' not in sys.path:
    sys.path.insert(0, '/opt/trn_rl_repo')

import numpy as np
import ml_dtypes

import concourse.bass as bass
import concourse.mybir as mybir
from concourse import bacc, tile
from concourse.bass import ts, ds
from concourse.bass_utils import run_bass_kernel_spmd
from concourse.masks import make_identity

B, S, D, H = 2, 2048, 1024, 16
HD = D // H                 # 64
NCORES = 8
HPC = H // NCORES           # 2 heads per core
SEQT = 128
NT = (B * S) // SEQT        # 32 seq tiles (batch-major)
TPB = S // SEQT             # 16 tiles per batch
QSPAN = 512
NSPAN = S // QSPAN          # 4 q-spans per batch
ROWS = (B * S) // NCORES    # 512 output rows per core
KC = D // 128               # 8 contraction chunks
W3 = 3 * HPC * HD           # 384 qkv columns per core
BF = mybir.dt.bfloat16
F32 = mybir.dt.float32
HALF_LN8 = 1.0397207708399179  # 0.5*ln(8): folds the 8.0 score scale
AF = mybir.ActivationFunctionType
MUL = mybir.AluOpType.mult


def build(dbg=False):
    nc = bacc.Bacc("TRN2", target_bir_lowering=False, debug=False,
                   num_devices=NCORES)
    xt = nc.dram_tensor("xt", [D, B * S], BF, kind="ExternalInput")
    wq = nc.dram_tensor("wq", [D, W3], BF, kind="ExternalInput")
    ba = nc.dram_tensor("ba", [1, W3], BF, kind="ExternalInput")
    wp = nc.dram_tensor("wp", [D, D], BF, kind="ExternalInput")
    bp = nc.dram_tensor("bp", [1, D], BF, kind="ExternalInput")
    out = nc.dram_tensor("out", [ROWS, D], F32, kind="ExternalOutput")
    if dbg:
        d_qt = nc.dram_tensor("d_qt", [128, HPC * S], BF, kind="ExternalOutput")
        d_kt = nc.dram_tensor("d_kt", [128, HPC * S], BF, kind="ExternalOutput")
        d_v = nc.dram_tensor("d_v", [128, NT * 2 * (HD + 1)], BF,
                             kind="ExternalOutput")
        d_oc = nc.dram_tensor("d_oc", [128, NCORES * ROWS], BF,
                              kind="ExternalOutput")

    with tile.TileContext(nc) as tc:
        with tc.tile_pool(name="persist", bufs=1) as pp, \
             tc.tile_pool(name="dram", bufs=1, space="DRAM") as dram, \
             tc.tile_pool(name="work", bufs=4) as work:

            # ---- persistent SBUF ----
            wq_sb = pp.tile([128, KC, W3], BF, name="wq_sb")
            wp_sb = pp.tile([128, KC, D], BF, name="wp_sb")
            ba_sb = pp.tile([1, W3], BF, name="ba_sb")
            bp_sb = pp.tile([1, D], BF, name="bp_sb")
            ones_sb = pp.tile([1, 128], BF, name="ones_sb")
            ones_f32 = pp.tile([1, HD], F32, name="ones_f32")
            c_bias = pp.tile([128, 1], F32, name="c_bias")
            c_scale = pp.tile([128, 1], F32, name="c_scale")
            ident = pp.tile([128, 128], BF, name="ident")
            tri = pp.tile([128, 128], BF, name="tri")
            # q,k working copies (normalized in place) + batched norm stats
            qk_all = pp.tile([128, NT, 2 * HPC * HD], BF, name="qk_all")
            n2_all = pp.tile([128, NT, 2 * HPC], F32, name="n2_all")
            rn_all = pp.tile([128, NT, 2 * HPC], F32, name="rn_all")
            # qT/kT per HEAD: batch0 rows 0:64, batch1 rows 64:128
            QT = [pp.tile([128, S], BF, name=f"qt{h}") for h in range(HPC)]
            KT = [pp.tile([128, S], BF, name=f"kt{h}") for h in range(HPC)]
            # v in [seq, hd] layout, per head augmented with a ones column
            v_sb = pp.tile([128, NT, 2 * (HD + 1)], BF, name="v_sb")
            ocT = pp.tile([128, NCORES, ROWS], BF, name="ocT")

            # per-head A2A payload is oT: shard p = rows [64p:64p+64] = this
            # core's head-h oT columns destined for core p
            a2a_in = [dram.tile([64 * NCORES, ROWS], BF, name=f"a2a_in{h}")
                      for h in range(HPC)]
            a2a_out = [dram.tile([64 * NCORES, ROWS], BF, name=f"a2a_out{h}")
                       for h in range(HPC)]

            # ---- constants (ordered so phase B can start early) ----
            for kc in range(KC):
                nc.sync.dma_start(wq_sb[:, kc, :], wq[ts(kc, 128), :])
            nc.sync.dma_start(ba_sb[:], ba[:])
            nc.sync.dma_start(bp_sb[:], bp[:])
            nc.gpsimd.memset(ones_sb[:], 1.0)
            nc.gpsimd.memset(ones_f32[:], 1.0)
            nc.gpsimd.memset(c_bias[:], HALF_LN8)
            nc.gpsimd.memset(c_scale[:], -0.5)
            nc.gpsimd.memset(v_sb[:], 1.0)
            make_identity(nc, ident[:])
            # tri[k, q] = 1 where q >= k (valid causal), else 0
            nc.gpsimd.memset(tri[:], 1.0)
            nc.gpsimd.affine_select(
                out=tri[:], in_=tri[:], compare_op=mybir.AluOpType.is_ge,
                fill=0.0, base=0, pattern=[[1, 128]], channel_multiplier=-1)

            # ---- phase B1: qkv matmuls + norm stats ----
            # xt lives only here; scoping frees its 64KB/partition for the
            # attention-phase e-tile pool
            with tc.tile_pool(name="xtp", bufs=1) as xtp, \
                 tc.tile_pool(name="ps_qkv", bufs=2, space="PSUM") as ps_qkv:
                xt_sb = xtp.tile([128, KC, B * S], BF, name="xt_sb")
                for tq in range(4):
                    for kc in range(KC):
                        nc.sync.dma_start(xt_sb[:, kc, ds(1024 * tq, 1024)],
                                          xt[ts(kc, 128), ds(1024 * tq, 1024)])
                for kc in range(KC):
                    nc.sync.dma_start(wp_sb[:, kc, :], wp[ts(kc, 128), :])
                for t in range(NT):
                    ps = ps_qkv.tile([128, W3], F32, tag="ps", name=f"ps{t}")
                    for kc in range(KC):
                        nc.tensor.matmul(ps[:], lhsT=xt_sb[:, kc, ts(t, 128)],
                                         rhs=wq_sb[:, kc, :],
                                         start=(kc == 0), stop=False)
                    nc.tensor.matmul(ps[:], lhsT=ones_sb[:, 0:128], rhs=ba_sb[:],
                                     start=False, stop=True)
                    nc.scalar.copy(qk_all[:, t, :], ps[:, 0:256])
                    nc.scalar.copy(
                        v_sb[:, t, :].rearrange(
                            "p (h e) -> p h e", e=HD + 1)[:, :, 0:HD],
                        ps[:, 256:384].rearrange("p (h e) -> p h e", e=HD))
                    sq = work.tile([128, 2 * HPC * HD], BF, tag="sq",
                                   name=f"sq{t}")
                    nc.vector.tensor_mul(sq[:], qk_all[:, t, :], qk_all[:, t, :])
                    nc.vector.reduce_sum(
                        n2_all[:, t, :], sq[:].rearrange("p (g e) -> p g e", e=HD),
                        axis=mybir.AxisListType.X)

            # batched rnorm = sqrt(8)/sqrt(n2): one Ln + one Exp (one table
            # set load each instead of per-tile thrash)
            nc.scalar.activation(rn_all[:].rearrange("p a b -> p (a b)"),
                                 n2_all[:].rearrange("p a b -> p (a b)"), AF.Ln)
            nc.scalar.activation(rn_all[:].rearrange("p a b -> p (a b)"),
                                 rn_all[:].rearrange("p a b -> p (a b)"), AF.Exp,
                                 scale=c_scale[:], bias=c_bias[:])

            with tc.tile_pool(name="ps_tr", bufs=2, space="PSUM") as ps_tr, \
                 tc.tile_pool(name="psB", bufs=2, space="PSUM") as psB, \
                 tc.tile_pool(name="psC", bufs=4, space="PSUM") as psC, \
                 tc.tile_pool(name="epool", bufs=6) as epool:

                # ---- phase B2: normalize + transpose into per-head
                # batch-packed QT/KT ----
                for t in range(NT):
                    b_, tt = divmod(t, TPB)
                    nc.vector.tensor_tensor(
                        qk_all[:, t, :].rearrange("p (g e) -> p g e", e=HD),
                        qk_all[:, t, :].rearrange("p (g e) -> p g e", e=HD),
                        rn_all[:, t, :, None].broadcast_to([128, 4, HD]), op=MUL)
                    for src0, dst in ((0, QT), (128, KT)):
                        trp = ps_tr.tile([128, 128], BF, tag="tr",
                                         name=f"tr{t}_{src0}")
                        nc.tensor.transpose(
                            trp[:], qk_all[:, t, src0:src0 + 128], ident[:])
                        # rows 0:64 = head0, 64:128 = head1 -> batch-packed
                        # per-head tensors (h0 copy on DVE, h1 on ACT)
                        nc.vector.tensor_copy(
                            dst[0][64 * b_:64 * b_ + 64, ts(tt, 128)],
                            trp[0:64, :])
                        nc.scalar.copy(
                            dst[1][64 * b_:64 * b_ + 64, ts(tt, 128)],
                            trp[64:128, :])

                # ---- phase C: attention (head-major for A2A pipelining;
                # batches row-tiled into concurrent PE row-groups; AV in
                # transposed form: one matmul per k-tile accumulating
                # oT+denom rows, so the result is directly the A2A payload)
                for h in range(HPC):
                    for j in range(NSPAN):
                        nk = 4 * j + 4
                        avT = [psC.tile([HD + 1, QSPAN], F32, tag="av",
                                        name=f"avT{h}_{j}_{b}")
                               for b in range(B)]
                        for i in range(nk):
                            d = i - 4 * j
                            c0 = max(d, 0)
                            for b_ in range(B):
                                sps = psB.tile([128, QSPAN], F32, tag="s",
                                               name=f"s{h}_{j}_{b_}_{i}")
                                nc.tensor.matmul(
                                    sps[:],
                                    lhsT=KT[h][64 * b_:64 * b_ + 64, ts(i, 128)],
                                    rhs=QT[h][64 * b_:64 * b_ + 64,
                                              ds(j * QSPAN, QSPAN)],
                                    start=True, stop=True)
                                e = epool.tile([128, QSPAN], BF, tag="e",
                                               name=f"e{h}_{j}_{b_}_{i}")
                                nc.scalar.activation(e[:, 128 * c0:],
                                                     sps[:, 128 * c0:], AF.Exp)
                                if d >= 0:
                                    nc.vector.tensor_tensor(
                                        e[:, 128 * d:128 * (d + 1)],
                                        e[:, 128 * d:128 * (d + 1)], tri[:],
                                        op=MUL)
                                nc.tensor.matmul(
                                    avT[b_][:, 128 * c0:],
                                    lhsT=v_sb[:, b_ * TPB + i,
                                              65 * h:65 * h + 65],
                                    rhs=e[:, 128 * c0:],
                                    start=(i == 0), stop=(i == nk - 1))
                        for b_ in range(B):
                            # denominator row -> SBUF, rank-1 broadcast into
                            # a PSUM bank, 64-lane reciprocal, one multiply
                            dcp = work.tile([1, QSPAN], F32, tag="dcp",
                                            name=f"dcp{h}_{j}_{b_}")
                            nc.vector.tensor_copy(dcp[:], avT[b_][HD:HD + 1, :])
                            rdb = ps_tr.tile([HD, QSPAN], F32, tag="tr",
                                             name=f"rdb{h}_{j}_{b_}")
                            nc.tensor.matmul(rdb[:], lhsT=ones_f32[:],
                                             rhs=dcp[:], start=True, stop=True)
                            rdbr = work.tile([HD, QSPAN], F32, tag="rdbr",
                                             name=f"rdbr{h}_{j}_{b_}")
                            nc.vector.reciprocal(rdbr[:], rdb[:])
                            ot = work.tile([HD, QSPAN], BF, tag="ot",
                                           name=f"ot{h}_{j}_{b_}")
                            nc.vector.tensor_tensor(ot[:], avT[b_][0:HD, :],
                                                    rdbr[:], op=MUL)
                            nc.sync.dma_start(
                                a2a_in[h][ts(4 * b_ + j, 64), :], ot[:])
                    # per-head A2A fires as soon as this head's o is out;
                    # head 0's collective overlaps head 1's attention
                    nc.gpsimd.collective_compute(
                        "AllToAll", mybir.AluOpType.bypass,
                        replica_groups=[list(range(NCORES))],
                        ins=[a2a_in[h][:].opt()], outs=[a2a_out[h][:].opt()])
                    nc.sync.dma_start(
                        ocT[64 * h:64 * h + 64, :, :],
                        a2a_out[h][:].rearrange("(p c) w -> c p w", c=64))

                if dbg:
                    for h in range(HPC):
                        nc.sync.dma_start(d_qt[:, ts(h, S)], QT[h][:])
                        nc.sync.dma_start(d_kt[:, ts(h, S)], KT[h][:])
                    nc.sync.dma_start(d_v[:],
                                      v_sb[:].rearrange("p a b -> p (a b)"))
                    nc.sync.dma_start(
                        d_oc[:], ocT[:].rearrange("p a b -> p (a b)"))

                # ---- phase D: projection ----
                for rt in range(ROWS // 128):
                    for half in range(2):
                        yps = psB.tile([128, 512], F32, tag="s",
                                       name=f"y{rt}_{half}")
                        for p in range(NCORES):
                            nc.tensor.matmul(
                                yps[:], lhsT=ocT[:, p, ts(rt, 128)],
                                rhs=wp_sb[:, p, ds(half * 512, 512)],
                                start=(p == 0), stop=False)
                        nc.tensor.matmul(yps[:], lhsT=ones_sb[:, 0:128],
                                         rhs=bp_sb[:, ds(half * 512, 512)],
                                         start=False, stop=True)
                        ysb = work.tile([128, 512], F32, tag="y", bufs=4,
                                        name=f"ysb{rt}_{half}")
                        nc.vector.tensor_copy(ysb[:], yps[:])
                        nc.sync.dma_start(
                            out[ts(rt, 128), ds(half * 512, 512)], ysb[:])

    nc.compile()
    return nc


_NC = None


def _get_nc():
    global _NC
    if _NC is None:
        _NC = build()
    return _NC


def make_in_maps(x, w_attn, b_attn, w_proj, b_proj):
    bf = ml_dtypes.bfloat16
    xt = np.ascontiguousarray(x.reshape(B * S, D).T).astype(bf)
    wp_ = np.ascontiguousarray(w_proj).astype(bf)
    bp_ = b_proj.reshape(1, D).astype(bf)
    in_maps = []
    for c in range(NCORES):
        sl = slice(128 * c, 128 * c + 128)
        wq_ = np.ascontiguousarray(np.concatenate(
            [w_attn[:, sl], w_attn[:, 1024:2048][:, sl],
             w_attn[:, 2048:3072][:, sl]], axis=1)).astype(bf)
        ba_ = np.concatenate(
            [b_attn[sl], b_attn[1024:2048][sl],
             b_attn[2048:3072][sl]]).reshape(1, W3).astype(bf)
        in_maps.append({"xt": xt, "wq": wq_, "ba": ba_, "wp": wp_, "bp": bp_})
    return in_maps


def gather_out(results):
    out = np.empty((B, S, D), np.float32)
    for c in range(NCORES):
        out[c // 4, ROWS * (c % 4):ROWS * (c % 4 + 1), :] = results[c]["out"]
    return out


def kernel(x, w_attn, b_attn, w_proj, b_proj):
    nc = _get_nc()
    in_maps = make_in_maps(np.asarray(x, np.float32), np.asarray(w_attn, np.float32),
                           np.asarray(b_attn, np.float32),
                           np.asarray(w_proj, np.float32),
                           np.asarray(b_proj, np.float32))
    res = run_bass_kernel_spmd(nc, in_maps, core_ids=list(range(NCORES)))
    return gather_out(res.results)
